# revision 1
# baseline (speedup 1.0000x reference)
"""Trainium2 Bass kernel for nn_BigramLMLinear (embedding lookup).

Math: out[b, t, :] = W[:, inputs[b, t]]  ==  W.T[inputs[b, t], :]
  W: [16384, 16384] f32, inputs: [8, 2048] int, out: [8, 2048, 16384] f32.

Strategy: data-parallel over the 8*2048 tokens — each of the 8 NeuronCores
handles one batch row (2048 tokens) and holds a full replica of WT = W.T
(pre-transposed on the host so each lookup is a contiguous 64 KiB row read).
Per core the kernel is pure DMA: 16 tiles x (indirect-gather 128 rows of
64 KiB HBM->SBUF, then one 8 MiB store SBUF->HBM). No collectives.
"""

import os

import numpy as np

V = 16384          # vocab (rows of WT) and embedding dim (cols)
B = 8              # batch rows == number of cores
T = 2048           # tokens per core
P = 128            # SBUF partitions
N_TILES = T // P   # 16
N_CORES = 8

_CACHE = {}
LAST_RESULTS = None  # BassKernelResults of the most recent run (for test harness)


def _build_nc():
    import concourse.bacc as bacc
    import concourse.bass as bass
    import concourse.mybir as mybir
    import concourse.tile as tile

    nc = bacc.Bacc("TRN2", target_bir_lowering=False, debug=False)

    ids_ext = nc.declare_dram_parameter("ids", [P, N_TILES], mybir.dt.int32, isOutput=False)
    wt_ext = nc.declare_dram_parameter("wt", [V, V], mybir.dt.float32, isOutput=False)
    out_ext = nc.declare_dram_parameter("out", [T, V], mybir.dt.float32, isOutput=True)

    with tile.TileContext(nc) as tc:
        with (
            tc.tile_pool(name="idp", bufs=1) as idpool,
            tc.tile_pool(name="rows", bufs=2) as pool,
        ):
            ids_sb = idpool.tile([P, N_TILES], mybir.dt.int32)
            nc.sync.dma_start(out=ids_sb[:], in_=ids_ext[:])
            for t in range(N_TILES):
                row_tile = pool.tile([P, V], mybir.dt.float32)
                nc.gpsimd.indirect_dma_start(
                    out=row_tile[:],
                    out_offset=None,
                    in_=wt_ext[:],
                    in_offset=bass.IndirectOffsetOnAxis(ap=ids_sb[:, t : t + 1], axis=0),
                )
                nc.sync.dma_start(out=out_ext[t * P : (t + 1) * P, :], in_=row_tile[:])
    nc.compile()
    return nc


def kernel(inputs: np.ndarray, W: np.ndarray) -> np.ndarray:
    global LAST_RESULTS
    from concourse.bass_utils import run_bass_kernel_spmd

    if "nc" not in _CACHE:
        _CACHE["nc"] = _build_nc()
    nc = _CACHE["nc"]

    ids = np.asarray(inputs).astype(np.int32)           # [B, T]
    assert ids.shape == (B, T)
    wt = np.ascontiguousarray(np.asarray(W).T)          # [V, V]; row i == W[:, i]

    in_maps = []
    for c in range(N_CORES):
        # column t of the [P, N_TILES] layout = tokens t*P .. t*P+127
        ids_c = np.ascontiguousarray(ids[c].reshape(N_TILES, P).T)
        in_maps.append({"ids": ids_c, "wt": wt})

    res = run_bass_kernel_spmd(nc, in_maps, core_ids=list(range(N_CORES)))
    LAST_RESULTS = res

    out = np.stack([res.results[c]["out"] for c in range(N_CORES)], axis=0)
    return out  # [B, T, V] f32
